# revision 39
# baseline (speedup 1.0000x reference)
"""Multi-head causal attention (B=2, S=2048, D=2048, H=16, HD=128) on 8 TRN2
NeuronCores.

Sharding: data-parallel over batch (2 groups of 4 cores) x tensor-parallel
over heads (4 heads per core).  Each core computes q/k/v projections for its
512 columns (4 heads), causal attention for those heads, and a partial
(contraction-sharded) wo product.  The 4 partial outputs per batch are summed
on the host (the "all-reduce after wo" of the sharding hint).

Everything on-chip is computed in transposed orientation:
  xT [d, s] (host pre-transposed), qT/kT [j, s], scores^T [t, s], out^T [j2, s]
so every matmul contraction lands on the partition axis with zero on-chip
transposes.  All matmul operands are bf16 (fp32 PSUM accumulation): bf16
halves LDWEIGHTS time (which otherwise caps the tensor engine below the
512-cycle streaming rate), halves HBM traffic, and lets all four weight
matrices live in SBUF for the whole kernel (loaded once, not per chunk).

All DRAM tensors are host-side pre-tiled to [128, *] partition-major layout
so every DMA moves multi-KB contiguous lines per partition with a handful of
dma_start instructions (dma_start issue costs ~0.7us of engine time each, so
many small DMAs are issue-rate-bound, not bandwidth-bound).

Softmax uses exp without max-subtraction (scores are O(5), exact in fp32)
with causal masking via a precomputed staircase mask post-exp (exact zeros,
matching the reference's exp(-1e9) == 0 underflow).  Causality is exploited
at 128-key-tile granularity: diagonal key tiles only stream the valid q
columns (partial-N matmuls).  Softmax denominators: full-width exp tiles are
summed on the vector engine and reduced with a single ones-matmul per
(chunk, head); only diagonal tiles use individual ones-matmuls.

Emission is interleaved at fine grain: attention of chunk c-1 (whose score ->
exp -> PV chain is latency-bound on the scalar engine) is woven between the
q/v projection matmuls of chunk c, so the in-order tensor queue always has
dependency-free projection work while exps drain.  DMA issue runs on the
sync engine (loads) and gpsimd (stores), keeping the scalar engine free for
exp.
"""

import numpy as np
import ml_dtypes

import concourse.bass as bass
import concourse.tile as tile
from concourse import bacc, mybir
from concourse.bass_utils import run_bass_kernel_spmd

B, S, D = 2, 2048, 2048
H, HD = 16, 128
P = 128
JL = 512          # local q/k/v columns per core (4 heads)
NH = 4            # heads per core
CHUNK = 512       # s-chunk
NCH = S // CHUNK  # 4
DT = D // P       # 16 d-tiles
NT = S // P       # 16 t-tiles
SCALE = 1.0 / float(np.sqrt(HD))
XW = DT * CHUNK   # 8192: one chunk of x / out, tiled
WW = DT * JL      # 8192: one qkv weight, tiled

F32 = mybir.dt.float32
F32R = mybir.dt.float32r
BF16 = mybir.dt.bfloat16


def build_kernel():
    nc = bacc.Bacc("TRN2", target_bir_lowering=False, debug=False, num_devices=8)
    # all pre-tiled [128, *]; see make_in_maps for layouts
    xT = nc.dram_tensor("xT", [P, NCH * XW], BF16, kind="ExternalInput").ap()
    wqT = nc.dram_tensor("wqT", [P, WW], BF16, kind="ExternalInput").ap()
    wkT = nc.dram_tensor("wkT", [P, WW], BF16, kind="ExternalInput").ap()
    wvT = nc.dram_tensor("wvT", [P, WW], BF16, kind="ExternalInput").ap()
    woT = nc.dram_tensor("woT", [P, NH * D], BF16, kind="ExternalInput").ap()
    outT = nc.dram_tensor("outT", [P, NCH * XW], BF16, kind="ExternalOutput").ap()

    with tile.TileContext(nc) as tc:
        with (
            tc.tile_pool(name="persist", bufs=1) as persist,
            tc.tile_pool(name="xt", bufs=2) as xt_pool,
            tc.tile_pool(name="qt", bufs=2) as qt_pool,
            tc.tile_pool(name="exp", bufs=10) as exp_pool,
            tc.tile_pool(name="ot", bufs=5) as ot_pool,
            tc.tile_pool(name="esp", bufs=8) as esp_pool,
            tc.tile_pool(name="esq", bufs=6) as esq_pool,
            tc.tile_pool(name="esum", bufs=6) as esum_pool,
            tc.tile_pool(name="small", bufs=2) as small_pool,
            tc.tile_pool(name="osb", bufs=1) as osb_pool,
            tc.tile_pool(name="ps_main", bufs=1, space="PSUM") as ps_main,
            tc.tile_pool(name="ps_s", bufs=3, space="PSUM") as ps_s,
            tc.tile_pool(name="ps_rs", bufs=1, space="PSUM") as ps_rs,
        ):
            # staircase mask: master[p, u] = 1.0 iff u - p - 384 >= 0 else 0.0
            # (f32 scratch borrows the output-staging slot, free at this point)
            master_f = osb_pool.tile([P, 896], F32, name="master_f", tag="ob")
            nc.gpsimd.memset(master_f[:], 1.0)
            nc.gpsimd.affine_select(
                out=master_f[:], in_=master_f[:], pattern=[[1, 896]],
                compare_op=mybir.AluOpType.is_ge, fill=0.0,
                base=-384, channel_multiplier=-1,
            )
            master = persist.tile([P, 896], BF16, name="master")
            nc.vector.tensor_copy(master[:], master_f[:])
            ones_f = persist.tile([P, 1], F32, name="ones_f")
            nc.vector.memset(ones_f[:], 1.0)
            ones = persist.tile([P, 1], BF16, name="ones")
            nc.vector.tensor_copy(ones[:], ones_f[:])
            ones_r = persist.tile([P, 1], F32R, name="ones_r")
            nc.vector.tensor_copy(ones_r[:], ones_f[:])

            # persistent weights (bf16, loaded once during chunk-0 work)
            wq_all = persist.tile([P, WW], BF16, name="wq")
            wk_all = persist.tile([P, WW], BF16, name="wk")
            wv_all = persist.tile([P, WW], BF16, name="wv")
            wo_all = persist.tile([P, NH * D], BF16, name="wo")
            kT_t = [persist.tile([P, S], BF16, name=f"kT{h}") for h in range(NH)]
            v_t = [persist.tile([P, JL], BF16, name=f"v{t}") for t in range(NT)]

            xt_cur = [None]   # xt tile of the chunk being projected
            qt_of = {}        # chunk -> qt tiles
            ots_of = {}       # chunk -> normalized per-head attention outputs

            # one PSUM bank shared by all softmax denominators; row-sliced so
            # two heads can accumulate concurrently in the round-robin tail
            rs2 = ps_rs.tile([1, CHUNK], F32, name="rs2", tag="rs")

            QW = 4 * CHUNK    # 2048: one DMA slice = 4 d-tiles

            def gen_kproj(c):
                ssl = slice(c * CHUNK, (c + 1) * CHUNK)
                ps_k = [ps_main.tile([P, CHUNK], F32, name=f"psk{j}", tag=f"pm{j}")
                        for j in range(4)]
                xt = xt_pool.tile([P, XW], BF16, name="xt", tag="xt")
                for d in range(DT):
                    if c == 0 and d < 2:
                        # tiny first slices, x on the scalar queue so wk and
                        # xt issue in parallel and the first matmul starts ASAP
                        sl = slice(d * CHUNK, CHUNK if d == 0 else QW)
                        nc.sync.dma_start(out=wk_all[:, sl], in_=wkT[:, sl])
                        nc.scalar.dma_start(
                            out=xt[:, sl], in_=xT[:, sl.start:sl.stop])
                    elif d % 4 == 0:
                        k = d // 4
                        qsl = slice(k * QW, (k + 1) * QW)
                        if c == 0:
                            nc.sync.dma_start(out=wk_all[:, qsl], in_=wkT[:, qsl])
                        nc.sync.dma_start(out=xt[:, qsl],
                                          in_=xT[:, c * XW + k * QW:
                                                 c * XW + (k + 1) * QW])
                    elif c == 0 and d % 4 == 2:
                        # wq prefetch trails the wk/xt slices so it never
                        # delays them on the shared queue
                        k = d // 4
                        qsl = slice(k * QW, (k + 1) * QW)
                        nc.sync.dma_start(out=wq_all[:, qsl], in_=wqT[:, qsl])
                    for j in range(4):
                        nc.tensor.matmul(
                            ps_k[j][:],
                            wk_all[:, d * JL + j * P:d * JL + (j + 1) * P],
                            xt[:, d * CHUNK:(d + 1) * CHUNK],
                            start=(d == 0), stop=(d == DT - 1),
                            skip_group_check=True,
                        )
                        if d == DT - 1:
                            # stagger: cast j overlaps the remaining matmuls
                            nc.vector.tensor_copy(kT_t[j][:, ssl], ps_k[j][:])
                    yield
                xt_cur[0] = xt
                yield

            def gen_qv(c):
                # qproj then vproj of chunk c; 33 yields
                xt = xt_cur[0]
                ps_q = [ps_main.tile([P, CHUNK], F32, name=f"psq{j}", tag=f"pm{j}")
                        for j in range(4)]
                qt = []
                for d in range(DT):
                    for j in range(4):
                        nc.tensor.matmul(
                            ps_q[j][:],
                            wq_all[:, d * JL + j * P:d * JL + (j + 1) * P],
                            xt[:, d * CHUNK:(d + 1) * CHUNK],
                            start=(d == 0), stop=(d == DT - 1),
                            skip_group_check=True,
                        )
                        if d == DT - 1:
                            t_ = qt_pool.tile([P, CHUNK], BF16, name=f"qt{j}",
                                              tag=f"qt{j}")
                            nc.vector.tensor_copy(t_[:], ps_q[j][:])
                            qt.append(t_)
                    if c == 0 and d % 4 == 0:
                        k = d // 4
                        qsl = slice(k * QW, (k + 1) * QW)
                        nc.sync.dma_start(out=wv_all[:, qsl], in_=wvT[:, qsl])
                    yield
                qt_of[c] = qt
                ps_v = [ps_main.tile([P, CHUNK], F32, name=f"psv{i}", tag=f"pm{i}")
                        for i in range(4)]
                for d in range(DT):
                    for i in range(4):
                        nc.tensor.matmul(
                            ps_v[i][:],
                            xt[:, d * CHUNK + i * P:d * CHUNK + (i + 1) * P],
                            wv_all[:, d * JL:(d + 1) * JL],
                            start=(d == 0), stop=(d == DT - 1),
                            skip_group_check=True,
                        )
                        if d == DT - 1:
                            nc.vector.tensor_copy(v_t[4 * c + i][:], ps_v[i][:])
                    if c == 0 and d % 8 == 0:
                        half = slice((d // 8) * 2 * D, ((d // 8) + 1) * 2 * D)
                        nc.gpsimd.dma_start(out=wo_all[:, half], in_=woT[:, half])
                    yield
                yield

            def make_head(c, h, otag="ss"):
                """Shared emission helpers for one attention head.

                Returns (emit_scores, emit_b, finish).  The softmax
                denominator for full-width key tiles is built on the vector
                engine as a bf16 pair/quad tree with an f32 top chain (one
                ones-matmul at the end instead of one per tile); diagonal
                tiles use ones-matmuls (non-tail) or in-place partial adds
                into the tree root (tail mode, so the rs PSUM window is
                short-lived and heads can round-robin).
                """
                qt = qt_of[c]
                T = 4 * c + 4
                NFULL = 4 * c
                all_esum = otag != "ss"
                opool = ps_s if otag == "ss" else ps_main
                o_acc = opool.tile([P, CHUNK], F32, name="oacc", tag=otag)
                exps = [None] * T
                st = {"pair": None, "quad": None, "esum": None}

                def emit_scores(t):
                    u = t - 4 * c
                    q0 = u * P if u > 0 else 0
                    ps = ps_s.tile([P, CHUNK], F32, name="pss", tag="ss")
                    nc.tensor.matmul(
                        ps[:, q0:], kT_t[h][:, t * P:(t + 1) * P], qt[h][:, q0:],
                        start=True, stop=True, skip_group_check=True,
                    )
                    e = exp_pool.tile([P, CHUNK], BF16, name="exp", tag="exp")
                    nc.scalar.activation(
                        e[:, q0:], ps[:, q0:], mybir.ActivationFunctionType.Exp,
                        scale=SCALE,
                    )
                    if u >= 0:
                        nc.vector.tensor_mul(
                            e[:, q0:q0 + P], e[:, q0:q0 + P], master[:, 384:384 + P])
                    exps[t] = (e, q0)
                    if u < 0:
                        if t % 2 == 1:
                            p_ = esp_pool.tile([P, CHUNK], BF16, name="esp",
                                               tag="esp")
                            nc.vector.tensor_add(p_[:], exps[t - 1][0][:], e[:])
                            if t % 4 == 1:
                                st["pair"] = p_
                            else:
                                q_ = esq_pool.tile([P, CHUNK], BF16, name="esq",
                                                   tag="esq")
                                nc.vector.tensor_add(q_[:], st["pair"][:], p_[:])
                                if t == 3:
                                    st["quad"] = q_
                                else:
                                    s_ = esum_pool.tile([P, CHUNK], F32R,
                                                        name="esum", tag="esum")
                                    if t == 7:
                                        nc.vector.tensor_add(
                                            s_[:], st["quad"][:], q_[:])
                                    else:
                                        nc.vector.tensor_add(
                                            s_[:], st["esum"][:], q_[:])
                                    st["esum"] = s_
                    elif all_esum:
                        # tail: fold diagonal tiles in-place into the root
                        nc.vector.tensor_add(
                            st["esum"][:, q0:], st["esum"][:, q0:], e[:, q0:])

                def esum_root():
                    # (tile, lhsT) for the final denominator matmul
                    if NFULL == 4:
                        return st["quad"], ones
                    return st["esum"], ones_r

                def emit_b(t):
                    e, q0 = exps[t]
                    if t >= NFULL and not all_esum:
                        nc.tensor.matmul(
                            rs2[:, q0:], ones[:], e[:, q0:],
                            start=(t == 0), stop=(t == T - 1),
                            skip_group_check=True,
                        )
                    nc.tensor.matmul(
                        o_acc[:, q0:], v_t[t][:, h * P:(h + 1) * P], e[:, q0:],
                        start=(t == 0), stop=(t == T - 1),
                        skip_group_check=True,
                    )

                def finish():
                    if all_esum:
                        root, lhs = esum_root()
                        nc.tensor.matmul(
                            rs2[:, :], lhs[:], root[:],
                            start=True, stop=True, skip_group_check=True,
                        )
                    rs_sb = small_pool.tile([1, CHUNK], F32, name="rssb",
                                            tag="rssb")
                    nc.vector.reciprocal_approx_fast(out=rs_sb[:], in_=rs2[:, :])
                    rb = small_pool.tile([P, CHUNK], F32, name="rb", tag="rb")
                    nc.gpsimd.partition_broadcast(rb[:], rs_sb[:])
                    ot = ot_pool.tile([P, CHUNK], BF16, name="ot", tag="ot")
                    nc.vector.tensor_mul(ot[:], o_acc[:], rb[:])
                    ots_of.setdefault(c, []).append(ot)

                return emit_scores, esum_root, emit_b, finish

            def gen_attn_head(c, h):
                # non-tail: group-structured, 4G+1 yields, G = c+1
                T = 4 * c + 4
                G = T // 4
                NFULL = 4 * c
                emit_scores, esum_root, emit_b, finish = make_head(c, h)
                for g in range(G):
                    emit_scores(4 * g); emit_scores(4 * g + 1)
                    yield
                    emit_scores(4 * g + 2); emit_scores(4 * g + 3)
                    yield
                    if g >= 1:
                        emit_b(4 * (g - 1)); emit_b(4 * (g - 1) + 1)
                        yield
                        emit_b(4 * (g - 1) + 2); emit_b(4 * (g - 1) + 3)
                        yield
                if NFULL > 0:
                    root, lhs = esum_root()
                    nc.tensor.matmul(
                        rs2[:, :], lhs[:], root[:],
                        start=True, stop=False, skip_group_check=True,
                    )
                emit_b(4 * (G - 1)); emit_b(4 * (G - 1) + 1)
                yield
                emit_b(4 * (G - 1) + 2); emit_b(4 * (G - 1) + 3)
                finish()
                yield

            def gen_attn(c):
                for h in range(NH):
                    yield from gen_attn_head(c, h)

            def gen_attn_head_tail(c, h):
                # tail: tile-granular (T+1 yields) for 4-way head round-robin
                T = 4 * c + 4
                emit_scores, _, emit_b, finish = make_head(c, h, otag=f"pm{h}")
                for t in range(T):
                    emit_scores(t)
                    if t >= 1:
                        emit_b(t - 1)
                    yield
                emit_b(T - 1)
                finish()
                yield

            def gen_wo(c, scalar_cast=False):
                ots = ots_of.pop(c)
                ob = osb_pool.tile([P, XW], BF16, name="ob", tag="ob")
                for j2 in range(DT):
                    pw = ps_s.tile([P, CHUNK], F32, name="pw", tag="ss")
                    for h in range(NH):
                        nc.tensor.matmul(
                            pw[:],
                            wo_all[:, h * D + j2 * P:h * D + (j2 + 1) * P],
                            ots[h][:],
                            start=(h == 0), stop=(h == NH - 1),
                            skip_group_check=True,
                        )
                    osl = slice(j2 * CHUNK, (j2 + 1) * CHUNK)
                    if scalar_cast and j2 % 2 == 0:
                        # split tail casts between scalar and vector
                        nc.scalar.activation(
                            ob[:, osl], pw[:],
                            mybir.ActivationFunctionType.Copy)
                    else:
                        nc.vector.tensor_copy(ob[:, osl], pw[:])
                    if j2 % 2 == 1:
                        osl = slice((j2 - 1) * CHUNK, (j2 + 1) * CHUNK)
                        nc.gpsimd.dma_start(
                            out=outT[:, c * XW + (j2 - 1) * CHUNK:
                                     c * XW + (j2 + 1) * CHUNK],
                            in_=ob[:, osl])
                    yield

            def drive(g):
                for _ in g:
                    pass

            SENT = object()

            def interleave(gmain, nmain, gsub, nsub):
                im = isub = 0
                main_done = sub_done = False
                while not (main_done and sub_done):
                    go_main = sub_done or (
                        not main_done and im * nsub <= isub * nmain)
                    if go_main:
                        if next(gmain, SENT) is SENT:
                            main_done = True
                        else:
                            im += 1
                    else:
                        if next(gsub, SENT) is SENT:
                            sub_done = True
                        else:
                            isub += 1

            def round_robin(gens):
                gens = list(gens)
                while gens:
                    done = []
                    for g in gens:
                        if next(g, SENT) is SENT:
                            done.append(g)
                    for g in done:
                        gens.remove(g)

            # ---- schedule ----
            drive(gen_kproj(0))
            drive(gen_qv(0))
            for c in range(1, NCH):
                drive(gen_kproj(c))
                interleave(gen_qv(c), 33, gen_attn(c - 1), 16 * c + 4)
                if c < NCH - 1:
                    drive(gen_wo(c - 1, scalar_cast=True))
            # tail: tile-granular round-robin over all 4 heads (one head's exp
            # latency hides behind the others' matmuls; o_acc in the idle
            # projection PSUM banks), with wo of the previous chunk woven in
            # as dependency-free tensor filler.
            cl = NCH - 1
            round_robin([gen_attn_head_tail(cl, h) for h in range(NH)]
                        + [gen_wo(cl - 1)])
            drive(gen_wo(cl, scalar_cast=True))

    nc.compile()
    return nc


_NC_CACHE = None


def _get_nc():
    global _NC_CACHE
    if _NC_CACHE is None:
        _NC_CACHE = build_kernel()
    return _NC_CACHE


def _tile128(a):
    """[R, C] -> [128, (R/128)*C] with out[p, r*C + c] = a[r*128 + p, c]."""
    R, C = a.shape
    return np.ascontiguousarray(
        a.reshape(R // P, P, C).transpose(1, 0, 2).reshape(P, -1))


def make_in_maps(x, wq, wk, wv, wo):
    bf16 = ml_dtypes.bfloat16
    in_maps = []
    for core in range(8):
        b, g = core // 4, core % 4
        j0 = g * JL
        # xT tiled [p, c, d, s']: = x[b][c*512+s', d*128+p]
        xb = x[b].astype(bf16)                      # [s, dcol]
        xt = np.ascontiguousarray(
            xb.reshape(NCH, CHUNK, DT, P).transpose(3, 0, 2, 1).reshape(P, -1))
        in_maps.append({
            "xT": xt,
            "wqT": _tile128(wq[j0:j0 + JL, :].T.astype(bf16)),
            "wkT": _tile128(wk[j0:j0 + JL, :].T.astype(bf16)),
            "wvT": _tile128(wv[j0:j0 + JL, :].T.astype(bf16)),
            "woT": _tile128(wo[:, j0:j0 + JL].T.astype(bf16)),
        })
    return in_maps


def kernel(x, freqs_complex=None, mask=None, wq=None, wk=None, wv=None, wo=None,
           **_unused):
    x = np.asarray(x, dtype=np.float32)
    wq = np.asarray(wq, dtype=np.float32)
    wk = np.asarray(wk, dtype=np.float32)
    wv = np.asarray(wv, dtype=np.float32)
    wo = np.asarray(wo, dtype=np.float32)

    nc = _get_nc()
    in_maps = make_in_maps(x, wq, wk, wv, wo)
    res = run_bass_kernel_spmd(nc, in_maps, list(range(8)))

    out = np.zeros((B, S, D), dtype=np.float32)
    for core in range(8):
        # outT tiled [p, c, j2, s'] -> out[b][c*512+s', j2*128+p]
        arr = np.asarray(res.results[core]["outT"]).astype(np.float32)
        out[core // 4] += arr.reshape(P, NCH, DT, CHUNK).transpose(
            1, 3, 2, 0).reshape(S, D)
    return out
